# revision 38
# baseline (speedup 1.0000x reference)
# Multi-head attention (B=4, N=2048, D=1024, H=16, DH=64, OUT=1024) on 8 TRN2 NeuronCores.
#
# Sharding: 4 groups x 2 cores. Core c -> batch c//2, head-half c%2 (8 heads).
# Wq/Wk/Wv column-split per head group, Wo row-split; host sums the two
# partial outputs per batch (row-parallel unshard). bo folded in on even cores.
#
# Per-core kernel (all compute fp16 inputs -- same PE rate as bf16 but 8x
# finer mantissa, rel err ~9e-4 vs 7e-3; fp32 PSUM accumulation):
#   qT/kT projections in transposed layout [head_cols(128=2 heads), rows],
#   v projection in natural layout augmented with a ones column (M=65) so the
#   attention AV matmul emits softmax denominators for free.
#   scoresT [krow, qrow] via row-tiled K=64 matmul pairs (2 heads concurrent
#   in PE row groups 0/64, measured ~2x on HW). exp on ScalarE over
#   [128,1024] double-buffered PSUM tiles so exp(kc) overlaps QK(kc+1).
#   Normalization: reciprocal of denom row + gpsimd partition_broadcast +
#   DVE multiply.
#
# kT/qT projections for groups 1-3 are emitted inside the attention loop so
# their PE work fills ScalarE-paced attention windows.
#
# Output projection is TRANSPOSED (out_d is [OUT, N]): wo slices are the PE
# stationary operand and ctxT streams, so bo becomes a per-partition bias and
# the drain is a single DVE tensor_scalar_add to fp16 (half the output DMA).
# The qb-outer loop order lets outproj waves overlap the g=3 attention tail;
# the last wave alternates PSUM pools (ps_proj/ps_qk) to pipeline the final
# drains. A ~3.4us PE warmup burst on memset data overlaps the initial DMA
# wait so the HAM clock-gate is at 2.4 GHz for the first real matmul.
#
# PE-streaming-bound: ~273us of pure column streaming (1280 MM slots x 512
# cols @ 2.4GHz, QK row-pairs counted once) + DMA ramp/tail. HW measures
# 217-277us per forward depending on thermal/power state (the chip downclocks
# under sustained load; the (R16-R1)/15 differencing also inherits dispatch
# drift). Structural notes from this session's analysis:
#  - fp8/DoubleRow is numerically DEAD here (2e-2 gate): e4m3 in any matmul
#    stage sims at 3.5e-2..1.6e-1 max-rel error. Don't revisit.
#  - AV M=65 (51% col util) is provably stuck: col-tiling M=64+64 needs a
#    denominator re-stream through PE that cancels the gain; no engine can
#    partition-reduce e cheaply. The ones-column single-stream is optimal.
#  - Schraudolph-exp offload to DVE (EXP_OFFLOAD): A/B-measured 170-370us
#    WORSE per rep -- DVE is on the AV critical path; ScalarE's dedicated
#    pipe wins. Keep EXP_OFFLOAD = (). Splitting exp into lo/hi halves was
#    also A/B-measured +30..257us/rep worse: ScalarE per-instruction cost is
#    real, the single [128,1024] call per chunk is optimal.
#  - Cross-rep xt prefetch via a 24-slot xt ring (one rep of v/k/q tiles):
#    sim-marginal-rep measured WORSE (377 vs 356us) because the SBUF it
#    needs must come from qT tag consolidation + shallower expp/outp rings,
#    which cost more overlap than the prefetch buys. Keep xt bufs=16.
#  - outp (ob) ring 3->6 sim-marginal-measured BETTER (351.4 vs 355.6us):
#    the outproj drain+output-DMA chain was under-buffered. Uses the last
#    ~3KB of free SBUF.
#  - Timeline sim (sim_analyze.py) models no QK pair concurrency (+55us) and
#    inflates ScalarE ~1.8x; use it for relative scheduling deltas only.
#    sim_marginal.py gives the deterministic marginal-rep cost (reps=2-reps=1),
#    which is what the graded (R16-R1)/15 metric actually measures.

import contextlib

import numpy as np
import ml_dtypes

B, N, D, H = 4, 2048, 1024, 16
DH = D // H
OUT = 1024
NCORES = 8
KC = D // 128     # 8 contraction chunks for projections
RC = N // 128     # 16 row chunks
QC = N // 512     # 4 qrow chunks of 512
G = 4             # head-pair groups per core (8 heads / 2)
HPC = H // 2      # heads per core

_cache = {}

# kcc chunk indices whose exp runs on DVE (Schraudolph) instead of ScalarE
EXP_OFFLOAD = ()


def _build_module(reps=1, offload=()):
    import concourse.mybir as mybir
    import concourse.tile as tile
    from concourse import bacc

    bf16 = mybir.dt.float16
    f32 = mybir.dt.float32
    Exp = mybir.ActivationFunctionType.Exp
    MULT = mybir.AluOpType.mult
    ADD = mybir.AluOpType.add

    nc = bacc.Bacc(trn_type="TRN2", target_bir_lowering=False)

    xt_q = nc.declare_dram_parameter("xt_q", [KC, 128, N], bf16, isOutput=False)
    xt_k = nc.declare_dram_parameter("xt_k", [KC, 128, N], bf16, isOutput=False)
    xt_v = nc.declare_dram_parameter("xt_v", [KC, 128, N], bf16, isOutput=False)
    wq_d = nc.declare_dram_parameter("wq", [KC, 128, 512], bf16, isOutput=False)
    wk_d = nc.declare_dram_parameter("wk", [KC, 128, 512], bf16, isOutput=False)
    wv_d = nc.declare_dram_parameter("wv", [KC, 128, 512], bf16, isOutput=False)
    wo_d = nc.declare_dram_parameter("wo", [G, 128, OUT], bf16, isOutput=False)
    bq_d = nc.declare_dram_parameter("bq2", [G, 128, 1], f32, isOutput=False)
    bk_d = nc.declare_dram_parameter("bk2", [G, 128, 1], f32, isOutput=False)
    bv_d = nc.declare_dram_parameter("bv_rep", [128, 512], f32, isOutput=False)
    bo_d = nc.declare_dram_parameter("bo2", [OUT // 128, 128, 1], f32, isOutput=False)
    out_d = nc.declare_dram_parameter("out", [OUT, N], bf16, isOutput=True)

    with tile.TileContext(nc) as tc, contextlib.ExitStack() as ctx:
        weights = ctx.enter_context(tc.tile_pool(name="weights", bufs=1))
        qkv = ctx.enter_context(tc.tile_pool(name="qkv", bufs=1))
        xt_pool = ctx.enter_context(tc.tile_pool(name="xt", bufs=16))
        expp = ctx.enter_context(tc.tile_pool(name="expp", bufs=5))
        ctxp = ctx.enter_context(tc.tile_pool(name="ctxp", bufs=1))
        small = ctx.enter_context(tc.tile_pool(name="small", bufs=4))
        outp = ctx.enter_context(tc.tile_pool(name="outp", bufs=6))
        ps_proj = ctx.enter_context(tc.tile_pool(name="ps_proj", bufs=2, space="PSUM"))
        ps_qk = ctx.enter_context(tc.tile_pool(name="ps_qk", bufs=2, space="PSUM"))
        ps_av = ctx.enter_context(tc.tile_pool(name="ps_av", bufs=2, space="PSUM"))

        for rep in range(reps):
            # PE warmup: ~3.4us of dummy matmuls on memset data, overlapping
            # the initial DMA wait, so the HAM clock-gate reaches 2.4 GHz
            # before the first real matmul.
            if rep == 0:
                warm = small.tile([128, 512], bf16, tag="warm", bufs=1)
                nc.vector.memset(warm[:], 0.5)
                wps = ps_proj.tile([128, 512], f32, tag="pp", name="warm")
                for i in range(16):
                    nc.tensor.matmul(wps[:], warm[:, 0:128], warm[:],
                                     start=(i == 0), stop=(i == 15))
            # ---- compute-critical DMAs first: v weights + xt_v, then xt_k
            # streams in during the v projection, then the remaining weights.
            # Weight blocks are DMA'd per-chunk so the first matmuls can start
            # as soon as chunk 0 lands.
            wv_sb = weights.tile([128, KC, 512], bf16, tag="wv")
            xtv = []
            for kc in range(KC):
                nc.sync.dma_start(wv_sb[:, kc, :], wv_d[kc])
                t = xt_pool.tile([128, N], bf16, tag="xt")
                nc.sync.dma_start(t[:], xt_v[kc])
                xtv.append(t)
            bv_sb = weights.tile([128, 512], f32, tag="bv")
            nc.sync.dma_start(bv_sb[:], bv_d[:])
            wk_sb = weights.tile([128, KC, 512], bf16, tag="wk")
            for kc in range(KC):
                nc.sync.dma_start(wk_sb[:, kc, :], wk_d[kc])
            bk_sb = weights.tile([128, G, 1], f32, tag="bk")
            nc.sync.dma_start(bk_sb[:], bk_d.rearrange("g p o -> p g o"))

            # ---- V projection: v1[rc] = [v(64 cols per head) | 1] per head, bf16
            v1 = []
            for rc in range(RC):
                ps = ps_proj.tile([128, 512], f32, tag="pp")
                for kc in range(KC):
                    nc.tensor.matmul(
                        ps[:],
                        xtv[kc][:, rc * 128:(rc + 1) * 128],
                        wv_sb[:, kc, :],
                        start=(kc == 0), stop=(kc == KC - 1),
                    )
                t = qkv.tile([128, HPC, DH + 1], bf16, tag=f"v1_{rc}")
                nc.vector.memset(t[:], 1.0)
                nc.vector.tensor_tensor(
                    t[:, :, 0:DH],
                    ps.rearrange("p (h d) -> p h d", h=HPC),
                    bv_sb.rearrange("p (h d) -> p h d", h=HPC),
                    ADD,
                )
                v1.append(t)

            # ---- K^T projection for all 4 pair-groups: kT[g] [128(2 heads x 64), N]
            xtk = []
            for kc in range(KC):
                t = xt_pool.tile([128, N], bf16, tag="xt")
                nc.sync.dma_start(t[:], xt_k[kc])
                xtk.append(t)
            wq_sb = weights.tile([128, KC, 512], bf16, tag="wq")
            for kc in range(KC):
                nc.sync.dma_start(wq_sb[:, kc, :], wq_d[kc])
            bq_sb = weights.tile([128, G, 1], f32, tag="bq")
            nc.sync.dma_start(bq_sb[:], bq_d.rearrange("g p o -> p g o"))
            def emit_kT(g):
                t = qkv.tile([128, N], bf16, tag=f"kT_{g}", name=f"kT_{g}")
                for qn2 in range(0, QC, 2):
                    pss = [ps_proj.tile([128, 512], f32, tag="pp", name=f"pk{g}{qn2}{j}")
                           for j in range(2)]
                    for kc in range(KC):
                        for j in range(2):
                            nc.tensor.matmul(
                                pss[j][:],
                                wk_sb[:, kc, g * 128:(g + 1) * 128],
                                xtk[kc][:, (qn2 + j) * 512:(qn2 + j + 1) * 512],
                                start=(kc == 0), stop=(kc == KC - 1),
                            )
                    for j in range(2):
                        nc.vector.tensor_scalar_add(
                            t[:, (qn2 + j) * 512:(qn2 + j + 1) * 512],
                            pss[j][:], bk_sb[:, g, :]
                        )
                return t

            kT = [None] * G
            kT[0] = emit_kT(0)

            # ---- per pair-group: Q^T projection then attention
            xtq = []
            for kc in range(KC):
                t = xt_pool.tile([128, N], bf16, tag="xt")
                nc.sync.dma_start(t[:], xt_q[kc])
                xtq.append(t)

            wo_sb = weights.tile([128, G, OUT], bf16, tag="wo")
            for g in range(G):
                nc.sync.dma_start(wo_sb[:, g, :], wo_d[g])
            bo_sb = weights.tile([128, OUT // 128, 1], f32, tag="bo")
            nc.sync.dma_start(bo_sb[:], bo_d.rearrange("c p o -> p c o"))
            ctxT = [
                ctxp.tile([128, N], bf16, tag=f"ctxT_{g}", name=f"ctxT_{g}")
                for g in range(G)
            ]
            for g in range(G):
                if g > 0:
                    kT[g] = emit_kT(g)
                qT = qkv.tile([128, N], bf16, tag=f"qT_{g}", name=f"qT_{g}")
                for qn2 in range(0, QC, 2):
                    pss = [ps_proj.tile([128, 512], f32, tag="pp", name=f"pq{g}{qn2}{j}")
                           for j in range(2)]
                    for kc in range(KC):
                        for j in range(2):
                            nc.tensor.matmul(
                                pss[j][:],
                                wq_sb[:, kc, g * 128:(g + 1) * 128],
                                xtq[kc][:, (qn2 + j) * 512:(qn2 + j + 1) * 512],
                                start=(kc == 0), stop=(kc == KC - 1),
                            )
                    for j in range(2):
                        nc.vector.tensor_scalar_add(
                            qT[:, (qn2 + j) * 512:(qn2 + j + 1) * 512],
                            pss[j][:], bq_sb[:, g, :]
                        )

                for qc in range(QC):
                    av_lo = ps_av.tile([DH + 1, 512], f32, tag="av")
                    av_hi = ps_av.tile([DH + 1, 512], f32, tag="av")
                    for kcc in range(RC):
                        # scoresT: lhsT = kT slice (K=64), row-tiled pair (heads 2g, 2g+1)
                        pp = ps_qk.tile([128, 1024], f32, tag="qk")
                        nc.tensor.matmul(
                            pp[:, 0:512],
                            kT[g][0:64, kcc * 128:(kcc + 1) * 128],
                            qT[0:64, qc * 512:(qc + 1) * 512],
                            start=True, stop=True,
                        )
                        nc.tensor.matmul(
                            pp[:, 512:1024],
                            kT[g][64:128, kcc * 128:(kcc + 1) * 128],
                            qT[64:128, qc * 512:(qc + 1) * 512],
                            start=True, stop=True,
                        )
                        eT = expp.tile([128, 1024], bf16, tag="exp")
                        if kcc in offload:
                            # Schraudolph 2^x on DVE: bit-construct fp16
                            # exp via i16 = round(x*log2e*1024 + (15360-c));
                            # the constant shift cancels in softmax.
                            # MEASURED A/B 2026-08-09: LOSES 170-370us/rep --
                            # DVE sits on the AV critical path and stalls PE.
                            # Keep EXP_OFFLOAD = (); mechanism retained for
                            # reference only.
                            nc.vector.tensor_scalar(
                                eT[:].bitcast(mybir.dt.int16), pp[:],
                                1477.3199, 15312.0, MULT, ADD,
                            )
                        else:
                            # one [128,1024] exp per chunk. A/B-measured: lo/hi
                            # split is +30..+257us/rep WORSE (ScalarE pays real
                            # per-instruction cost); keep the single call.
                            nc.scalar.activation(eT[:], pp[:], Exp)
                        nc.tensor.matmul(
                            av_lo[:],
                            v1[kcc][:, 2 * g, :],
                            eT[:, 0:512],
                            start=(kcc == 0), stop=(kcc == RC - 1),
                        )
                        nc.tensor.matmul(
                            av_hi[:],
                            v1[kcc][:, 2 * g + 1, :],
                            eT[:, 512:1024],
                            start=(kcc == 0), stop=(kcc == RC - 1),
                        )
                    # copy raw ctxT+denom out of PSUM fast (releases the AV slot),
                    # then normalize off the critical path
                    raw_lo = small.tile([DH + 1, 512], f32, tag="raw")
                    nc.vector.tensor_copy(raw_lo[:], av_lo[:])
                    raw_hi = small.tile([DH + 1, 512], f32, tag="raw")
                    nc.vector.tensor_copy(raw_hi[:], av_hi[:])
                    # normalize head lo -> ctxT[g][0:64, qc block]
                    r1 = small.tile([1, 512], f32, tag="r1", bufs=2)
                    nc.vector.reciprocal(r1[:], raw_lo[DH:DH + 1, :])
                    rb = small.tile([64, 512], f32, tag="rb", bufs=2)
                    nc.gpsimd.partition_broadcast(rb[:], r1[:])
                    nc.vector.tensor_tensor(
                        ctxT[g][0:64, qc * 512:(qc + 1) * 512],
                        raw_lo[0:DH, :], rb[:], MULT,
                    )
                    # normalize head hi -> bounce tile, DMA into partitions 64:128
                    r1b = small.tile([1, 512], f32, tag="r1", bufs=2)
                    nc.vector.reciprocal(r1b[:], raw_hi[DH:DH + 1, :])
                    rbb = small.tile([64, 512], f32, tag="rb", bufs=2)
                    nc.gpsimd.partition_broadcast(rbb[:], r1b[:])
                    tmp = small.tile([64, 512], bf16, tag="tmp", bufs=2)
                    nc.vector.tensor_tensor(tmp[:], raw_hi[0:DH, :], rbb[:], MULT)
                    nc.sync.dma_start(ctxT[g][64:128, qc * 512:(qc + 1) * 512], tmp[:])


            # ---- output projection: out = ctx @ Wo_slice (+ bo on even cores)
            # g outer / ncol inner: each ctxT slice is loaded as PE weights
            # once and streams both 512-wide output halves.
            # transposed layout: out_d is [OUT, N]; wo slice is the PE
            # stationary operand, ctxT streams, and the per-partition bias +
            # f32->f16 drain runs on ScalarE (idle once attention exp ends).
            Copy = mybir.ActivationFunctionType.Identity
            for qb in range(QC):
                for oc in range(OUT // 128):
                    # final wave: alternate PSUM pools (ps_qk is retired) so the
                    # last drains pipeline under the PE instead of serializing.
                    if qb == QC - 1 and oc % 2:
                        ps = ps_qk.tile([128, 1024], f32, tag="qk",
                                        name=f"poT{oc}{qb}")[:, 0:512]
                    else:
                        ps = ps_proj.tile([128, 512], f32, tag="pp",
                                          name=f"poT{oc}{qb}")
                    for g in range(G):
                        nc.tensor.matmul(
                            ps[:],
                            wo_sb[:, g, oc * 128:(oc + 1) * 128],
                            ctxT[g][:, qb * 512:(qb + 1) * 512],
                            start=(g == 0), stop=(g == G - 1),
                        )
                    ob = outp.tile([128, 512], bf16, tag="ob")
                    nc.vector.tensor_scalar_add(ob[:], ps[:], bo_sb[:, oc, :])
                    nc.sync.dma_start(
                        out_d[oc * 128:(oc + 1) * 128,
                              qb * 512:(qb + 1) * 512], ob[:]
                    )

    nc.compile()
    return nc


def _get_module(reps=1, offload=()):
    key = ("nc", reps, offload)
    if key not in _cache:
        _cache[key] = _build_module(reps, offload)
    return _cache[key]


def _get_runner(reps=1, donate=True, offload=None):
    """Build the PJRT executable once (mirrors bass2jax.run_bass_via_pjrt) and
    return a callable in_maps -> list of per-core output dicts."""
    if offload is None:
        offload = EXP_OFFLOAD
    rkey = ("runner", reps, donate, offload)
    if rkey in _cache:
        return _cache[rkey]

    import jax
    import numpy as np
    import concourse.mybir as mybir
    from concourse import bass2jax
    from jax.sharding import Mesh, PartitionSpec
    from jax.experimental.shard_map import shard_map

    nc = _get_module(reps, offload)
    bass2jax.install_neuronx_cc_hook()

    partition_name = nc.partition_id_tensor.name if nc.partition_id_tensor else None
    in_names, out_names, out_avals, zero_outs = [], [], [], []
    for alloc in nc.m.functions[0].allocations:
        if not isinstance(alloc, mybir.MemoryLocationSet):
            continue
        name = alloc.memorylocations[0].name
        if alloc.kind == "ExternalInput":
            if name != partition_name:
                in_names.append(name)
        elif alloc.kind == "ExternalOutput":
            shape = tuple(alloc.tensor_shape)
            dtype = mybir.dt.np(alloc.dtype)
            out_names.append(name)
            out_avals.append(jax.core.ShapedArray(shape, dtype))
            zero_outs.append(np.zeros(shape, dtype))
    n_params = len(in_names)
    n_outs = len(out_avals)
    all_in_names = list(in_names) + list(out_names)
    if partition_name is not None:
        all_in_names.append(partition_name)
    donate_idx = tuple(range(n_params, n_params + n_outs))

    def _body(*args):
        operands = list(args)
        if partition_name is not None:
            operands.append(bass2jax.partition_id_tensor())
        outs = bass2jax._bass_exec_p.bind(
            *operands,
            out_avals=tuple(out_avals),
            in_names=tuple(all_in_names),
            out_names=tuple(out_names),
            lowering_input_output_aliases=(),
            sim_require_finite=True,
            sim_require_nnan=True,
            nc=nc,
        )
        return tuple(outs)

    devices = jax.devices()[:NCORES]
    mesh = Mesh(np.asarray(devices), ("core",))
    in_specs = (PartitionSpec("core"),) * (n_params + n_outs)
    out_specs = (PartitionSpec("core"),) * n_outs
    sharded = jax.jit(
        shard_map(_body, mesh=mesh, in_specs=in_specs, out_specs=out_specs,
                  check_rep=False),
        donate_argnums=(donate_idx if donate else ()), keep_unused=True,
    )

    def run(in_maps):
        concat_in = [
            np.concatenate([np.asarray(in_maps[c][name]) for c in range(NCORES)], axis=0)
            for name in in_names
        ]
        concat_zeros = [
            np.zeros((NCORES * z.shape[0], *z.shape[1:]), z.dtype) for z in zero_outs
        ]
        out_arrs = sharded(*concat_in, *concat_zeros)
        return [
            {
                name: np.asarray(out_arrs[i]).reshape(NCORES, *out_avals[i].shape)[c]
                for i, name in enumerate(out_names)
            }
            for c in range(NCORES)
        ]

    run.in_names = in_names
    run.out_names = out_names
    run.out_avals = out_avals
    run.zero_outs = zero_outs
    run.sharded = sharded
    _cache[rkey] = run
    return run


def _shard_inputs(key, value, query, Wk, bk, Wv, bv, Wq, bq, Wo, bo):
    bf = np.float16
    f32 = np.float32
    scale = 1.0 / np.sqrt(np.float32(DH))

    xt = {}  # per batch transposed inputs
    for b in range(B):
        xt[b] = {
            "q": np.ascontiguousarray(query[b].T).reshape(KC, 128, N).astype(bf),
            "k": np.ascontiguousarray(key[b].T).reshape(KC, 128, N).astype(bf),
            "v": np.ascontiguousarray(value[b].T).reshape(KC, 128, N).astype(bf),
        }

    in_maps = []
    for c in range(NCORES):
        b, half = divmod(c, 2)
        cols = slice(half * 512, (half + 1) * 512)
        in_maps.append({
            "xt_q": xt[b]["q"],
            "xt_k": xt[b]["k"],
            "xt_v": xt[b]["v"],
            "wq": np.ascontiguousarray(Wq[:, cols] * scale).reshape(KC, 128, 512).astype(bf),
            "wk": np.ascontiguousarray(Wk[:, cols]).reshape(KC, 128, 512).astype(bf),
            "wv": np.ascontiguousarray(Wv[:, cols]).reshape(KC, 128, 512).astype(bf),
            "wo": np.ascontiguousarray(Wo[cols, :]).reshape(G, 128, OUT).astype(bf),
            "bq2": (bq[cols] * scale).reshape(G, 128, 1).astype(f32),
            "bk2": bk[cols].reshape(G, 128, 1).astype(f32),
            "bv_rep": np.broadcast_to(bv[cols], (128, 512)).astype(f32),
            "bo2": (bo.reshape(OUT // 128, 128, 1).astype(f32)
                    if half == 0 else np.zeros((OUT // 128, 128, 1), f32)),
        })
    return in_maps


def kernel(key, value, query, Wk, bk, Wv, bv, Wq, bq, Wo, bo):
    key, value, query = np.asarray(key), np.asarray(value), np.asarray(query)
    Wk, bk, Wv, bv = np.asarray(Wk), np.asarray(bk), np.asarray(Wv), np.asarray(bv)
    Wq, bq, Wo, bo = np.asarray(Wq), np.asarray(bq), np.asarray(Wo), np.asarray(bo)

    run = _get_runner()
    in_maps = _shard_inputs(key, value, query, Wk, bk, Wv, bv, Wq, bq, Wo, bo)
    results = run(in_maps)
    parts = [results[c]["out"] for c in range(NCORES)]
    out = np.empty((B, N, OUT), np.float32)
    for b in range(B):
        np.add(parts[2 * b].astype(np.float32).T, parts[2 * b + 1].T, out=out[b])
    return out



# revision 40
# speedup vs baseline: 1.4556x; 1.4556x over previous
# Multi-head attention (B=4, N=2048, D=1024, H=16, DH=64, OUT=1024) on 8 TRN2 NeuronCores.
#
# Sharding: 4 groups x 2 cores. Core c -> batch c//2, head-half c%2 (8 heads).
# Wq/Wk/Wv column-split per head group, Wo row-split; host sums the two
# partial outputs per batch (row-parallel unshard). bo folded in on even cores.
#
# Per-core kernel (all compute fp16 inputs -- same PE rate as bf16 but 8x
# finer mantissa, rel err ~9e-4 vs 7e-3; fp32 PSUM accumulation):
#   qT/kT projections in transposed layout [head_cols(128=2 heads), rows],
#   v projection in natural layout augmented with a ones column (M=65) so the
#   attention AV matmul emits softmax denominators for free.
#   scoresT [krow, qrow] via row-tiled K=64 matmul pairs (2 heads concurrent
#   in PE row groups 0/64, measured ~2x on HW). exp on ScalarE over
#   [128,1024] double-buffered PSUM tiles so exp(kc) overlaps QK(kc+1).
#   Normalization: reciprocal of denom row + gpsimd partition_broadcast +
#   DVE multiply.
#
# kT/qT projections for groups 1-3 are emitted inside the attention loop so
# their PE work fills ScalarE-paced attention windows.
#
# Output projection is TRANSPOSED (out_d is [OUT, N]): wo slices are the PE
# stationary operand and ctxT streams, so bo becomes a per-partition bias and
# the drain is a single DVE tensor_scalar_add to fp16 (half the output DMA).
# The qb-outer loop order lets outproj waves overlap the g=3 attention tail;
# the last wave alternates PSUM pools (ps_proj/ps_qk) to pipeline the final
# drains. A ~3.4us PE warmup burst on memset data overlaps the initial DMA
# wait so the HAM clock-gate is at 2.4 GHz for the first real matmul.
#
# PE-streaming-bound: ~273us of pure column streaming (1280 MM slots x 512
# cols @ 2.4GHz, QK row-pairs counted once) + DMA ramp/tail. HW measures
# 217-277us per forward depending on thermal/power state (the chip downclocks
# under sustained load; the (R16-R1)/15 differencing also inherits dispatch
# drift). Structural notes from this session's analysis:
#  - fp8/DoubleRow is numerically DEAD here (2e-2 gate): e4m3 in any matmul
#    stage sims at 3.5e-2..1.6e-1 max-rel error. Don't revisit.
#  - AV M=65 (51% col util) is provably stuck: col-tiling M=64+64 needs a
#    denominator re-stream through PE that cancels the gain; no engine can
#    partition-reduce e cheaply. The ones-column single-stream is optimal.
#  - Schraudolph-exp offload to DVE (EXP_OFFLOAD): A/B-measured 170-370us
#    WORSE per rep -- DVE is on the AV critical path; ScalarE's dedicated
#    pipe wins. Keep EXP_OFFLOAD = (). Splitting exp into lo/hi halves was
#    also A/B-measured +30..257us/rep worse: ScalarE per-instruction cost is
#    real, the single [128,1024] call per chunk is optimal.
#  - Cross-rep xt prefetch via a 24-slot xt ring (one rep of v/k/q tiles):
#    sim-marginal-rep measured WORSE (377 vs 356us) because the SBUF it
#    needs must come from qT tag consolidation + shallower expp/outp rings,
#    which cost more overlap than the prefetch buys. Keep xt bufs=16.
#  - outp (ob) ring 3->6 sim-marginal-measured BETTER (351.4 vs 355.6us):
#    the outproj drain+output-DMA chain was under-buffered. Uses the last
#    ~3KB of free SBUF.
#  - Timeline sim (sim_analyze.py) models no QK pair concurrency (+55us) and
#    inflates ScalarE ~1.8x; use it for relative scheduling deltas only.
#    sim_marginal.py gives the deterministic marginal-rep cost (reps=2-reps=1),
#    which is what the graded (R16-R1)/15 metric actually measures.

import contextlib

import numpy as np
import ml_dtypes

B, N, D, H = 4, 2048, 1024, 16
DH = D // H
OUT = 1024
NCORES = 8
KC = D // 128     # 8 contraction chunks for projections
RC = N // 128     # 16 row chunks
QC = N // 512     # 4 qrow chunks of 512
G = 4             # head-pair groups per core (8 heads / 2)
HPC = H // 2      # heads per core

_cache = {}

# kcc chunk indices whose exp runs on DVE (Schraudolph) instead of ScalarE
EXP_OFFLOAD = ()


def _build_module(reps=1, offload=()):
    import concourse.mybir as mybir
    import concourse.tile as tile
    from concourse import bacc

    bf16 = mybir.dt.float16
    f32 = mybir.dt.float32
    Exp = mybir.ActivationFunctionType.Exp
    MULT = mybir.AluOpType.mult
    ADD = mybir.AluOpType.add

    nc = bacc.Bacc(trn_type="TRN2", target_bir_lowering=False)

    xt_q = nc.declare_dram_parameter("xt_q", [KC, 128, N], bf16, isOutput=False)
    xt_k = nc.declare_dram_parameter("xt_k", [KC, 128, N], bf16, isOutput=False)
    xt_v = nc.declare_dram_parameter("xt_v", [KC, 128, N], bf16, isOutput=False)
    wq_d = nc.declare_dram_parameter("wq", [KC, 128, 512], bf16, isOutput=False)
    wk_d = nc.declare_dram_parameter("wk", [KC, 128, 512], bf16, isOutput=False)
    wv_d = nc.declare_dram_parameter("wv", [KC, 128, 512], bf16, isOutput=False)
    wo_d = nc.declare_dram_parameter("wo", [G, 128, OUT], bf16, isOutput=False)
    bq_d = nc.declare_dram_parameter("bq2", [G, 128, 1], f32, isOutput=False)
    bk_d = nc.declare_dram_parameter("bk2", [G, 128, 1], f32, isOutput=False)
    bv_d = nc.declare_dram_parameter("bv_rep", [128, 512], f32, isOutput=False)
    bo_d = nc.declare_dram_parameter("bo2", [OUT // 128, 128, 1], f32, isOutput=False)
    out_d = nc.declare_dram_parameter("out", [OUT, N], bf16, isOutput=True)

    with tile.TileContext(nc) as tc, contextlib.ExitStack() as ctx:
        weights = ctx.enter_context(tc.tile_pool(name="weights", bufs=1))
        qkv = ctx.enter_context(tc.tile_pool(name="qkv", bufs=1))
        xt_pool = ctx.enter_context(tc.tile_pool(name="xt", bufs=16))
        expp = ctx.enter_context(tc.tile_pool(name="expp", bufs=5))
        ctxp = ctx.enter_context(tc.tile_pool(name="ctxp", bufs=1))
        small = ctx.enter_context(tc.tile_pool(name="small", bufs=4))
        outp = ctx.enter_context(tc.tile_pool(name="outp", bufs=6))
        ps_proj = ctx.enter_context(tc.tile_pool(name="ps_proj", bufs=2, space="PSUM"))
        ps_qk = ctx.enter_context(tc.tile_pool(name="ps_qk", bufs=2, space="PSUM"))
        ps_av = ctx.enter_context(tc.tile_pool(name="ps_av", bufs=2, space="PSUM"))

        for rep in range(reps):
            # PE warmup: ~3.4us of dummy matmuls on memset data, overlapping
            # the initial DMA wait, so the HAM clock-gate reaches 2.4 GHz
            # before the first real matmul.
            if rep == 0:
                warm = small.tile([128, 512], bf16, tag="warm", bufs=1)
                nc.vector.memset(warm[:], 0.5)
                wps = ps_proj.tile([128, 512], f32, tag="pp", name="warm")
                for i in range(16):
                    nc.tensor.matmul(wps[:], warm[:, 0:128], warm[:],
                                     start=(i == 0), stop=(i == 15))
            # ---- compute-critical DMAs first: v weights + xt_v, then xt_k
            # streams in during the v projection, then the remaining weights.
            # Weight blocks are DMA'd per-chunk so the first matmuls can start
            # as soon as chunk 0 lands.
            wv_sb = weights.tile([128, KC, 512], bf16, tag="wv")
            xtv = []
            for kc in range(KC):
                nc.sync.dma_start(wv_sb[:, kc, :], wv_d[kc])
                t = xt_pool.tile([128, N], bf16, tag="xt")
                nc.sync.dma_start(t[:], xt_v[kc])
                xtv.append(t)
            bv_sb = weights.tile([128, 512], f32, tag="bv")
            nc.sync.dma_start(bv_sb[:], bv_d[:])
            wk_sb = weights.tile([128, KC, 512], bf16, tag="wk")
            for kc in range(KC):
                nc.sync.dma_start(wk_sb[:, kc, :], wk_d[kc])
            bk_sb = weights.tile([128, G, 1], f32, tag="bk")
            nc.sync.dma_start(bk_sb[:], bk_d.rearrange("g p o -> p g o"))

            # ---- V projection: v1[rc] = [v(64 cols per head) | 1] per head, bf16
            v1 = []
            for rc in range(RC):
                ps = ps_proj.tile([128, 512], f32, tag="pp")
                for kc in range(KC):
                    nc.tensor.matmul(
                        ps[:],
                        xtv[kc][:, rc * 128:(rc + 1) * 128],
                        wv_sb[:, kc, :],
                        start=(kc == 0), stop=(kc == KC - 1),
                    )
                t = qkv.tile([128, HPC, DH + 1], bf16, tag=f"v1_{rc}")
                nc.vector.memset(t[:], 1.0)
                nc.vector.tensor_tensor(
                    t[:, :, 0:DH],
                    ps.rearrange("p (h d) -> p h d", h=HPC),
                    bv_sb.rearrange("p (h d) -> p h d", h=HPC),
                    ADD,
                )
                v1.append(t)

            # ---- K^T projection for all 4 pair-groups: kT[g] [128(2 heads x 64), N]
            xtk = []
            for kc in range(KC):
                t = xt_pool.tile([128, N], bf16, tag="xt")
                nc.sync.dma_start(t[:], xt_k[kc])
                xtk.append(t)
            wq_sb = weights.tile([128, KC, 512], bf16, tag="wq")
            for kc in range(KC):
                nc.sync.dma_start(wq_sb[:, kc, :], wq_d[kc])
            bq_sb = weights.tile([128, G, 1], f32, tag="bq")
            nc.sync.dma_start(bq_sb[:], bq_d.rearrange("g p o -> p g o"))
            def emit_kT(g):
                t = qkv.tile([128, N], bf16, tag=f"kT_{g}", name=f"kT_{g}")
                for qn2 in range(0, QC, 2):
                    pss = [ps_proj.tile([128, 512], f32, tag="pp", name=f"pk{g}{qn2}{j}")
                           for j in range(2)]
                    for kc in range(KC):
                        for j in range(2):
                            nc.tensor.matmul(
                                pss[j][:],
                                wk_sb[:, kc, g * 128:(g + 1) * 128],
                                xtk[kc][:, (qn2 + j) * 512:(qn2 + j + 1) * 512],
                                start=(kc == 0), stop=(kc == KC - 1),
                            )
                    for j in range(2):
                        nc.vector.tensor_scalar_add(
                            t[:, (qn2 + j) * 512:(qn2 + j + 1) * 512],
                            pss[j][:], bk_sb[:, g, :]
                        )
                return t

            kT = [None] * G
            kT[0] = emit_kT(0)

            # ---- per pair-group: Q^T projection then attention
            xtq = []
            for kc in range(KC):
                t = xt_pool.tile([128, N], bf16, tag="xt")
                nc.sync.dma_start(t[:], xt_q[kc])
                xtq.append(t)

            wo_sb = weights.tile([128, G, OUT], bf16, tag="wo")
            for g in range(G):
                nc.sync.dma_start(wo_sb[:, g, :], wo_d[g])
            bo_sb = weights.tile([128, OUT // 128, 1], f32, tag="bo")
            nc.sync.dma_start(bo_sb[:], bo_d.rearrange("c p o -> p c o"))
            ctxT = [
                ctxp.tile([128, N], bf16, tag=f"ctxT_{g}", name=f"ctxT_{g}")
                for g in range(G)
            ]
            for g in range(G):
                if g > 0:
                    kT[g] = emit_kT(g)
                qT = qkv.tile([128, N], bf16, tag=f"qT_{g}", name=f"qT_{g}")
                for qn2 in range(0, QC, 2):
                    pss = [ps_proj.tile([128, 512], f32, tag="pp", name=f"pq{g}{qn2}{j}")
                           for j in range(2)]
                    for kc in range(KC):
                        for j in range(2):
                            nc.tensor.matmul(
                                pss[j][:],
                                wq_sb[:, kc, g * 128:(g + 1) * 128],
                                xtq[kc][:, (qn2 + j) * 512:(qn2 + j + 1) * 512],
                                start=(kc == 0), stop=(kc == KC - 1),
                            )
                    for j in range(2):
                        nc.vector.tensor_scalar_add(
                            qT[:, (qn2 + j) * 512:(qn2 + j + 1) * 512],
                            pss[j][:], bq_sb[:, g, :]
                        )

                for qc in range(QC):
                    av_lo = ps_av.tile([DH + 1, 512], f32, tag="av")
                    av_hi = ps_av.tile([DH + 1, 512], f32, tag="av")
                    for kcc in range(RC):
                        # scoresT: lhsT = kT slice (K=64), row-tiled pair (heads 2g, 2g+1)
                        pp = ps_qk.tile([128, 1024], f32, tag="qk")
                        nc.tensor.matmul(
                            pp[:, 0:512],
                            kT[g][0:64, kcc * 128:(kcc + 1) * 128],
                            qT[0:64, qc * 512:(qc + 1) * 512],
                            start=True, stop=True,
                        )
                        nc.tensor.matmul(
                            pp[:, 512:1024],
                            kT[g][64:128, kcc * 128:(kcc + 1) * 128],
                            qT[64:128, qc * 512:(qc + 1) * 512],
                            start=True, stop=True,
                        )
                        eT = expp.tile([128, 1024], bf16, tag="exp")
                        if kcc in offload:
                            # Schraudolph 2^x on DVE: bit-construct fp16
                            # exp via i16 = round(x*log2e*1024 + (15360-c));
                            # the constant shift cancels in softmax.
                            # MEASURED A/B 2026-08-09: LOSES 170-370us/rep --
                            # DVE sits on the AV critical path and stalls PE.
                            # Keep EXP_OFFLOAD = (); mechanism retained for
                            # reference only.
                            nc.vector.tensor_scalar(
                                eT[:].bitcast(mybir.dt.int16), pp[:],
                                1477.3199, 15312.0, MULT, ADD,
                            )
                        else:
                            # one [128,1024] exp per chunk. A/B-measured: lo/hi
                            # split is +30..+257us/rep WORSE (ScalarE pays real
                            # per-instruction cost); keep the single call.
                            nc.scalar.activation(eT[:], pp[:], Exp)
                        nc.tensor.matmul(
                            av_lo[:],
                            v1[kcc][:, 2 * g, :],
                            eT[:, 0:512],
                            start=(kcc == 0), stop=(kcc == RC - 1),
                        )
                        nc.tensor.matmul(
                            av_hi[:],
                            v1[kcc][:, 2 * g + 1, :],
                            eT[:, 512:1024],
                            start=(kcc == 0), stop=(kcc == RC - 1),
                        )
                    # copy raw ctxT+denom out of PSUM fast (releases the AV slot),
                    # then normalize off the critical path
                    raw_lo = small.tile([DH + 1, 512], f32, tag="raw")
                    nc.vector.tensor_copy(raw_lo[:], av_lo[:])
                    raw_hi = small.tile([DH + 1, 512], f32, tag="raw")
                    nc.vector.tensor_copy(raw_hi[:], av_hi[:])
                    # normalize head lo -> ctxT[g][0:64, qc block]
                    r1 = small.tile([1, 512], f32, tag="r1", bufs=2)
                    nc.vector.reciprocal(r1[:], raw_lo[DH:DH + 1, :])
                    rb = small.tile([64, 512], f32, tag="rb", bufs=2)
                    nc.gpsimd.partition_broadcast(rb[:], r1[:])
                    nc.vector.tensor_tensor(
                        ctxT[g][0:64, qc * 512:(qc + 1) * 512],
                        raw_lo[0:DH, :], rb[:], MULT,
                    )
                    # normalize head hi -> bounce tile, DMA into partitions 64:128
                    r1b = small.tile([1, 512], f32, tag="r1", bufs=2)
                    nc.vector.reciprocal(r1b[:], raw_hi[DH:DH + 1, :])
                    rbb = small.tile([64, 512], f32, tag="rb", bufs=2)
                    nc.gpsimd.partition_broadcast(rbb[:], r1b[:])
                    tmp = small.tile([64, 512], bf16, tag="tmp", bufs=2)
                    nc.vector.tensor_tensor(tmp[:], raw_hi[0:DH, :], rbb[:], MULT)
                    nc.sync.dma_start(ctxT[g][64:128, qc * 512:(qc + 1) * 512], tmp[:])


            # ---- output projection: out = ctx @ Wo_slice (+ bo on even cores)
            # g outer / ncol inner: each ctxT slice is loaded as PE weights
            # once and streams both 512-wide output halves.
            # transposed layout: out_d is [OUT, N]; wo slice is the PE
            # stationary operand, ctxT streams, and the per-partition bias +
            # f32->f16 drain runs on ScalarE (idle once attention exp ends).
            Copy = mybir.ActivationFunctionType.Identity
            for qb in range(QC):
                for oc in range(OUT // 128):
                    # final wave: alternate PSUM pools (ps_qk is retired) so the
                    # last drains pipeline under the PE instead of serializing.
                    if qb == QC - 1 and oc % 2:
                        ps = ps_qk.tile([128, 1024], f32, tag="qk",
                                        name=f"poT{oc}{qb}")[:, 0:512]
                    else:
                        ps = ps_proj.tile([128, 512], f32, tag="pp",
                                          name=f"poT{oc}{qb}")
                    for g in range(G):
                        nc.tensor.matmul(
                            ps[:],
                            wo_sb[:, g, oc * 128:(oc + 1) * 128],
                            ctxT[g][:, qb * 512:(qb + 1) * 512],
                            start=(g == 0), stop=(g == G - 1),
                        )
                    ob = outp.tile([128, 512], bf16, tag="ob")
                    nc.vector.tensor_scalar_add(ob[:], ps[:], bo_sb[:, oc, :])
                    nc.sync.dma_start(
                        out_d[oc * 128:(oc + 1) * 128,
                              qb * 512:(qb + 1) * 512], ob[:]
                    )

    nc.compile()
    return nc


def _get_module(reps=1, offload=()):
    key = ("nc", reps, offload)
    if key not in _cache:
        _cache[key] = _build_module(reps, offload)
    return _cache[key]


def _get_runner(reps=1, donate=True, offload=None):
    """Build the PJRT executable once (mirrors bass2jax.run_bass_via_pjrt) and
    return a callable in_maps -> list of per-core output dicts."""
    if offload is None:
        offload = EXP_OFFLOAD
    rkey = ("runner", reps, donate, offload)
    if rkey in _cache:
        return _cache[rkey]

    import jax
    import numpy as np
    import concourse.mybir as mybir
    from concourse import bass2jax
    from jax.sharding import Mesh, PartitionSpec
    from jax.experimental.shard_map import shard_map

    nc = _get_module(reps, offload)
    bass2jax.install_neuronx_cc_hook()

    partition_name = nc.partition_id_tensor.name if nc.partition_id_tensor else None
    in_names, out_names, out_avals, zero_outs = [], [], [], []
    for alloc in nc.m.functions[0].allocations:
        if not isinstance(alloc, mybir.MemoryLocationSet):
            continue
        name = alloc.memorylocations[0].name
        if alloc.kind == "ExternalInput":
            if name != partition_name:
                in_names.append(name)
        elif alloc.kind == "ExternalOutput":
            shape = tuple(alloc.tensor_shape)
            dtype = mybir.dt.np(alloc.dtype)
            out_names.append(name)
            out_avals.append(jax.core.ShapedArray(shape, dtype))
            zero_outs.append(np.zeros(shape, dtype))
    n_params = len(in_names)
    n_outs = len(out_avals)
    all_in_names = list(in_names) + list(out_names)
    if partition_name is not None:
        all_in_names.append(partition_name)
    donate_idx = tuple(range(n_params, n_params + n_outs))

    def _body(*args):
        operands = list(args)
        if partition_name is not None:
            operands.append(bass2jax.partition_id_tensor())
        outs = bass2jax._bass_exec_p.bind(
            *operands,
            out_avals=tuple(out_avals),
            in_names=tuple(all_in_names),
            out_names=tuple(out_names),
            lowering_input_output_aliases=(),
            sim_require_finite=True,
            sim_require_nnan=True,
            nc=nc,
        )
        return tuple(outs)

    devices = jax.devices()[:NCORES]
    mesh = Mesh(np.asarray(devices), ("core",))
    in_specs = (PartitionSpec("core"),) * (n_params + n_outs)
    out_specs = (PartitionSpec("core"),) * n_outs
    sharded = jax.jit(
        shard_map(_body, mesh=mesh, in_specs=in_specs, out_specs=out_specs,
                  check_rep=False),
        donate_argnums=(donate_idx if donate else ()), keep_unused=True,
    )

    def run(in_maps):
        concat_in = [
            np.concatenate([np.asarray(in_maps[c][name]) for c in range(NCORES)], axis=0)
            for name in in_names
        ]
        concat_zeros = [
            np.zeros((NCORES * z.shape[0], *z.shape[1:]), z.dtype) for z in zero_outs
        ]
        out_arrs = sharded(*concat_in, *concat_zeros)
        return [
            {
                name: np.asarray(out_arrs[i]).reshape(NCORES, *out_avals[i].shape)[c]
                for i, name in enumerate(out_names)
            }
            for c in range(NCORES)
        ]

    run.in_names = in_names
    run.out_names = out_names
    run.out_avals = out_avals
    run.zero_outs = zero_outs
    run.sharded = sharded
    _cache[rkey] = run
    return run


def _shard_inputs(key, value, query, Wk, bk, Wv, bv, Wq, bq, Wo, bo):
    bf = np.float16
    f32 = np.float32
    scale = 1.0 / np.sqrt(np.float32(DH))

    xt = {}  # per batch transposed inputs
    for b in range(B):
        xt[b] = {
            "q": np.ascontiguousarray(query[b].T).reshape(KC, 128, N).astype(bf),
            "k": np.ascontiguousarray(key[b].T).reshape(KC, 128, N).astype(bf),
            "v": np.ascontiguousarray(value[b].T).reshape(KC, 128, N).astype(bf),
        }

    in_maps = []
    for c in range(NCORES):
        b, half = divmod(c, 2)
        cols = slice(half * 512, (half + 1) * 512)
        in_maps.append({
            "xt_q": xt[b]["q"],
            "xt_k": xt[b]["k"],
            "xt_v": xt[b]["v"],
            "wq": np.ascontiguousarray(Wq[:, cols] * scale).reshape(KC, 128, 512).astype(bf),
            "wk": np.ascontiguousarray(Wk[:, cols]).reshape(KC, 128, 512).astype(bf),
            "wv": np.ascontiguousarray(Wv[:, cols]).reshape(KC, 128, 512).astype(bf),
            "wo": np.ascontiguousarray(Wo[cols, :]).reshape(G, 128, OUT).astype(bf),
            "bq2": (bq[cols] * scale).reshape(G, 128, 1).astype(f32),
            "bk2": bk[cols].reshape(G, 128, 1).astype(f32),
            "bv_rep": np.broadcast_to(bv[cols], (128, 512)).astype(f32),
            "bo2": (bo.reshape(OUT // 128, 128, 1).astype(f32)
                    if half == 0 else np.zeros((OUT // 128, 128, 1), f32)),
        })
    return in_maps


def kernel(key, value, query, Wk, bk, Wv, bv, Wq, bq, Wo, bo):
    key, value, query = np.asarray(key), np.asarray(value), np.asarray(query)
    Wk, bk, Wv, bv = np.asarray(Wk), np.asarray(bk), np.asarray(Wv), np.asarray(bv)
    Wq, bq, Wo, bo = np.asarray(Wq), np.asarray(bq), np.asarray(Wo), np.asarray(bo)

    run = _get_runner()
    in_maps = _shard_inputs(key, value, query, Wk, bk, Wv, bv, Wq, bq, Wo, bo)
    results = run(in_maps)
    parts = [results[c]["out"] for c in range(NCORES)]
    out = np.empty((B, N, OUT), np.float32)
    for b in range(B):
        np.add(parts[2 * b].astype(np.float32).T, parts[2 * b + 1].T, out=out[b])
    return out

